# revision 12
# baseline (speedup 1.0000x reference)
"""BatchSiren Trainium2 kernel.

B=2048 independent SIREN MLPs (2->32->32->3, sin activations, w0=30),
each evaluated on the same N=1024 coordinate grid.

Strategy (pure data parallel over 8 cores, 256 nets/core):
- 16 supergroups of 16 nets per core. Nets packed 16-at-a-time onto the
  128x128 PE array via 32x32 tile_position (independent concurrent tiles),
  fp32 throughout.
- All activations stay in [feature-partition, points-free] layout.
- sin(w0*z): weights pre-scaled by w0/2pi on host so matmuls produce
  arguments in CYCLE units; range reduction to one period via the
  magic-number round trick (ACT Identity+MAGIC / DVE tensor_scalar), then
  ACT Sin with scale=-2pi maps back to radians (Sin table covers [-pi,pi]).
- Layer-3 output is produced point-major ([points, 4nets x 3ch]) via
  col-tiled matmuls with block-diagonal w3, so the final DMA writes
  2KB-contiguous runs per partition.
"""
import numpy as np

import concourse.bacc as bacc
import concourse.bass as bass
import concourse.mybir as mybir
import concourse.tile as tile
from concourse import bass_utils

f32 = mybir.dt.float32
AF = mybir.ActivationFunctionType
ALU = mybir.AluOpType

W0 = 30.0
MAGIC = float(1.5 * 2 ** 23)
TWO_PI = float(2.0 * np.pi)
N_CORES = 8
B, N, IN, H, OUT = 2048, 1024, 2, 32, 3
BPC = B // N_CORES        # 256 nets per core
SGS = BPC // 16           # 16 supergroups of 16 nets
NH = N // 2               # 512 points per half

_compiled = None


def _build_module():
    nc = bacc.Bacc("TRN2", target_bir_lowering=False, debug=False)

    d_w1 = nc.dram_tensor("w1aug", [4, 3, 128 * SGS], f32, kind="ExternalInput")
    d_w2 = nc.dram_tensor("w2s", [4, 32, 128 * SGS], f32, kind="ExternalInput")
    d_w3 = nc.dram_tensor("w3blk", [4, 32, 48 * SGS], f32, kind="ExternalInput")
    d_sm = nc.dram_tensor("smalls", [128, 9 * SGS], f32, kind="ExternalInput")
    d_c = nc.dram_tensor("coords", [4, 3, N], f32, kind="ExternalInput")
    d_out = nc.dram_tensor("out", [SGS, 2, 48, NH], f32, kind="ExternalOutput")

    with tile.TileContext(nc) as tc:
        with tc.tile_pool(name="const", bufs=1) as cp, \
             tc.tile_pool(name="acts", bufs=2) as ap, \
             tc.tile_pool(name="outp", bufs=4) as op_, \
             tc.tile_pool(name="psA", bufs=4, space="PSUM") as psA:

            # ---- persistent constants (w2 chunked so sg0 starts early) ----
            w1sb = cp.tile([128, 128 * SGS], f32, tag="w1")
            for a in range(4):
                nc.sync.dma_start(w1sb[32 * a:32 * a + 3, :], d_w1[a])
            w2sb = cp.tile([128, 128 * SGS], f32, tag="w2")
            q = 128 * SGS // 4
            for b in range(4):
                for ck in range(4):
                    nc.sync.dma_start(
                        w2sb[32 * b:32 * b + 32, q * ck:q * ck + q],
                        d_w2[b, :, q * ck:q * ck + q])
            w3sb = cp.tile([128, 48 * SGS], f32, tag="w3")
            for a in range(4):
                nc.sync.dma_start(w3sb[32 * a:32 * a + 32, :], d_w3[a])
            smalls = cp.tile([128, 9 * SGS], f32, tag="sm")
            nc.sync.dma_start(smalls[:], d_sm[:])
            c4 = cp.tile([128, N], f32, tag="c4")
            nc.vector.memset(c4[:], 0.0)  # rows 3-31 of each group MUST be 0
            for a in range(4):
                nc.sync.dma_start(c4[32 * a:32 * a + 3, :], d_c[a])
            magic = cp.tile([128, 1], f32, tag="mg")
            nc.vector.memset(magic[:], MAGIC)

            def front_A(sg, h):
                # ---- layer 1: 16 nets as 32x32 tiles, K=3 (w,b aug) ----
                PZ1a = psA.tile([128, 1024], f32, tag="P")
                PZ1b = psA.tile([128, 1024], f32, tag="P")
                PZ1 = [PZ1a, PZ1b]
                for a in range(4):
                    for b in range(4):
                        nc.tensor.matmul(
                            out=PZ1[a // 2][32 * b:32 * b + 32,
                                            512 * (a % 2):512 * (a % 2) + 512],
                            lhsT=w1sb[32 * a:32 * a + 3,
                                      128 * sg + 32 * b:128 * sg + 32 * b + 32],
                            rhs=c4[32 * a:32 * a + 3, NH * h:NH * h + NH],
                            start=True, stop=True,
                            tile_position=(32 * a, 32 * b))
                Q1 = ap.tile([128, 2048], f32, tag="Q1")
                for g in range(2):
                    sl = slice(1024 * g, 1024 * g + 1024)
                    nc.scalar.activation(Q1[:, sl], PZ1[g][:], AF.Identity,
                                         bias=magic[:], scale=1.0)
                    nc.vector.scalar_tensor_tensor(
                        Q1[:, sl], Q1[:, sl], MAGIC, PZ1[g][:],
                        ALU.subtract, ALU.subtract)
                H1 = ap.tile([128, 2048], f32, tag="H1")
                nc.scalar.activation(H1[:], Q1[:], AF.Sin,
                                     bias=0.0, scale=-TWO_PI)
                return H1

            def front_B1(sg, h, H1):
                # ---- layer 2: 16 nets as 32x32 tiles, K=32 ----
                PZ2a = psA.tile([128, 1024], f32, tag="P")
                PZ2b = psA.tile([128, 1024], f32, tag="P")
                PZ2 = [PZ2a, PZ2b]
                for bt in range(4):
                    for a in range(4):
                        nc.tensor.matmul(
                            out=PZ2[bt // 2][32 * a:32 * a + 32,
                                             512 * (bt % 2):512 * (bt % 2) + 512],
                            lhsT=w2sb[32 * bt:32 * bt + 32,
                                      128 * sg + 32 * a:128 * sg + 32 * a + 32],
                            rhs=H1[32 * bt:32 * bt + 32,
                                   512 * a:512 * a + 512],
                            start=True, stop=True,
                            tile_position=(32 * bt, 32 * a))
                return PZ2

            def front_B2(sg, h, Q2, PZ2):
                for bt in range(4):
                    nc.vector.tensor_scalar(
                        Q2[:, 1024 * bt + 512 * h:1024 * bt + 512 * h + 512],
                        PZ2[bt // 2][:, 512 * (bt % 2):512 * (bt % 2) + 512],
                        smalls[:, 9 * sg + bt:9 * sg + bt + 1], MAGIC,
                        ALU.add, ALU.add)
                for g in range(2):
                    q2v = Q2[:].rearrange("p (b g2 n) -> p b g2 n",
                                          b=4, g2=2)[:, 2 * g:2 * g + 2, h, :]
                    z2v = PZ2[g][:].rearrange("p (b n) -> p b n", b=2)
                    nc.vector.scalar_tensor_tensor(
                        q2v, q2v, MAGIC, z2v, ALU.subtract, ALU.subtract)

            def back_sin2(sg, Q2, H2, bts=(0, 1, 2, 3)):
                for bt in bts:
                    nc.scalar.activation(
                        H2[:, 1024 * bt:1024 * bt + 1024],
                        Q2[:, 1024 * bt:1024 * bt + 1024],
                        AF.Sin,
                        bias=smalls[:, 9 * sg + 4 + bt:9 * sg + 4 + bt + 1],
                        scale=-TWO_PI)

            def back_out(sg, h, H2):
                PC = psA.tile([128, 512], f32, tag="P")
                for bt in range(4):
                    nc.tensor.matmul(
                        out=PC[32 * bt:32 * bt + 12, :],
                        lhsT=w3sb[:, 48 * sg + 12 * bt:48 * sg + 12 * bt + 12],
                        rhs=H2[:, 1024 * bt + 512 * h:1024 * bt + 512 * h + 512],
                        start=True, stop=True,
                        tile_position=(0, 32 * bt))
                OT = op_.tile([128, 512], f32, tag="OT")
                nc.scalar.activation(OT[:], PC[:], AF.Identity,
                                     bias=smalls[:, 9 * sg + 8:9 * sg + 9],
                                     scale=1.0)
                for bt in range(4):
                    nc.sync.dma_start(
                        d_out[sg, h, 12 * bt:12 * bt + 12, :],
                        OT[32 * bt:32 * bt + 12, :])

            # software-pipelined emission with deferred DVE tail:
            # per half t: L1(t) | B2(t-1) | back(sg-1) | Jt1/Jq1(t) | sin1(t) | L2(t)
            prev = None          # (sg, Q2, H2) of previous supergroup
    
            pending = None       # deferred front_B2 args of previous half
            for sg in range(SGS):
                Q2 = ap.tile([128, 4096], f32, tag="Q2")
                H2 = ap.tile([128, 4096], f32, tag="H2")
                for h in range(2):
                    H1 = front_A(sg, h)
                    if pending is not None:
                        front_B2(*pending)
                        pending = None
                    if h == 0 and prev is not None:
                        back_sin2(prev[0], prev[1], prev[2])
                    if prev is not None:
                        back_out(prev[0], h, prev[2])
                    PZ2 = front_B1(sg, h, H1)
                    pending = (sg, h, Q2, PZ2)
                prev = (sg, Q2, H2)
            front_B2(*pending)
            back_sin2(prev[0], prev[1], prev[2])
            back_out(prev[0], 0, prev[2])
            back_out(prev[0], 1, prev[2])

    nc.compile()
    return nc


def _prep_core_inputs(w1, b1, w2, b2, w3, b3, coords, core):
    s = np.float32(W0 / TWO_PI)
    B0 = core * BPC
    sl = slice(B0, B0 + BPC)

    # net (sg, a, b) = batch B0 + 16sg + 4a + b
    w1c = w1[sl, :, :, 0].reshape(SGS, 4, 4, IN, H)
    b1c = b1[sl, :, 0].reshape(SGS, 4, 4, H)
    aug = np.concatenate([w1c, b1c[:, :, :, None, :]], axis=3) * s  # [sg,a,b,3,32]
    w1aug = np.ascontiguousarray(
        aug.transpose(1, 3, 0, 2, 4).reshape(4, 3, SGS * 128)).astype(np.float32)

    # L2: partition 32bt+i, free 32a+o  (net 4a+bt)
    w2c = (w2[sl, :, :, 0] * s).reshape(SGS, 4, 4, H, H)  # [sg,a,b,i,o]
    w2s = np.ascontiguousarray(
        w2c.transpose(2, 3, 0, 1, 4).reshape(4, 32, SGS * 128)).astype(np.float32)

    # L3 block-diag per (sg, bt): [128, 12]: [32a+i, 3a'+c] = w3[net(sg,a,bt)]
    w3c = w3[sl, :, :, 0].reshape(SGS, 4, 4, H, OUT)  # [sg,a,b,i,c]
    blk = np.zeros((SGS, 4, 4, H, 4, OUT), np.float32)  # [sg,a,b,i,a',c]
    for a in range(4):
        blk[:, a, :, :, a, :] = w3c[:, a]
    # free inside sg block: 12*bt + 3*a' + c ; partition 32*a + i
    w3blk = np.ascontiguousarray(
        blk.transpose(1, 3, 0, 2, 4, 5).reshape(4, 32, SGS * 48)).astype(np.float32)

    b2c = b2[sl, :, 0].reshape(SGS, 4, 4, H)  # [sg,a,b,o]
    b3c = b3[sl, :, 0].reshape(SGS, 4, 4, OUT)  # [sg,a,b,c]
    smalls = np.zeros((128, SGS, 9), np.float32)
    p = np.arange(128)
    a_idx, o_idx = p // 32, p % 32
    for bt in range(4):
        # L2 psum partition 32a+o, segment bt -> net 4a+bt
        smalls[:, :, bt] = (b2c[:, a_idx, bt, o_idx] * s).T
        smalls[:, :, 4 + bt] = (b2c[:, a_idx, bt, o_idx] * np.float32(W0)).T
    # b3: PC partition 32bt + 3a + c -> net 4a+bt
    bt_idx, m_idx = p // 32, p % 32
    a3, c3 = m_idx // 3, m_idx % 3
    for pi in range(128):
        if m_idx[pi] < 12:
            smalls[pi, :, 8] = b3c[:, a3[pi], bt_idx[pi], c3[pi]]
    smalls = np.ascontiguousarray(smalls.reshape(128, SGS * 9))

    ch = np.zeros((4, 3, N), np.float32)
    ch[:, :IN, :] = coords.T[None, :, :]
    ch[:, IN, :] = 1.0

    return {"w1aug": w1aug, "w2s": w2s, "w3blk": w3blk,
            "smalls": smalls, "coords": ch}


def _unshard(res_list):
    outs = []
    for r in res_list:
        o = r["out"].reshape(SGS, 2, 4, 4, OUT, NH)      # [sg,h,bt,a,c,n]
        o = o.transpose(0, 3, 2, 1, 5, 4)                # [sg,a,bt,h,n,c]
        outs.append(np.ascontiguousarray(o.reshape(BPC, N, OUT)))
    return np.concatenate(outs, axis=0)


def _run(inputs, trace=False, trace_kwargs=None):
    global _compiled
    if _compiled is None:
        _compiled = _build_module()
    nc = _compiled
    arrs = {k: np.asarray(v, dtype=np.float32) for k, v in inputs.items()}
    in_maps = [_prep_core_inputs(arrs["w1"], arrs["b1"], arrs["w2"], arrs["b2"],
                                 arrs["w3"], arrs["b3"], arrs["coords"], c)
               for c in range(N_CORES)]
    kw = {}
    if trace:
        kw["trace"] = True
        if trace_kwargs:
            kw.update(trace_kwargs)
    res = bass_utils.run_bass_kernel_spmd(nc, in_maps, core_ids=list(range(N_CORES)),
                                          **kw)
    out = _unshard(res.results)
    return out, res


def kernel(**inputs):
    out, _ = _run(inputs, trace=False)
    return out


# revision 13
# speedup vs baseline: 1.0457x; 1.0457x over previous
"""BatchSiren Trainium2 kernel.

B=2048 independent SIREN MLPs (2->32->32->3, sin activations, w0=30),
each evaluated on the same N=1024 coordinate grid.

Strategy (pure data parallel over 8 cores, 256 nets/core):
- 16 supergroups of 16 nets per core. Nets packed 16-at-a-time onto the
  128x128 PE array via 32x32 tile_position (independent concurrent tiles),
  fp32 throughout.
- All activations stay in [feature-partition, points-free] layout.
- sin(w0*z): weights pre-scaled by w0/2pi on host so matmuls produce
  arguments in CYCLE units; range reduction to one period via the
  magic-number round trick (ACT Identity+MAGIC / DVE tensor_scalar), then
  ACT Sin with scale=-2pi maps back to radians (Sin table covers [-pi,pi]).
- Layer-3 output is produced point-major ([points, 4nets x 3ch]) via
  col-tiled matmuls with block-diagonal w3, so the final DMA writes
  2KB-contiguous runs per partition.
"""
import numpy as np

import concourse.bacc as bacc
import concourse.bass as bass
import concourse.mybir as mybir
import concourse.tile as tile
from concourse import bass_utils

f32 = mybir.dt.float32
AF = mybir.ActivationFunctionType
ALU = mybir.AluOpType

W0 = 30.0
MAGIC = float(1.5 * 2 ** 23)
TWO_PI = float(2.0 * np.pi)
N_CORES = 8
B, N, IN, H, OUT = 2048, 1024, 2, 32, 3
BPC = B // N_CORES        # 256 nets per core
SGS = BPC // 16           # 16 supergroups of 16 nets
NH = N // 2               # 512 points per half

_compiled = None


def _build_module():
    nc = bacc.Bacc("TRN2", target_bir_lowering=False, debug=False)

    d_w1 = nc.dram_tensor("w1aug", [4, 3, 128 * SGS], f32, kind="ExternalInput")
    d_w2 = nc.dram_tensor("w2s", [4, 32, 128 * SGS], f32, kind="ExternalInput")
    d_w3 = nc.dram_tensor("w3blk", [4, 32, 48 * SGS], f32, kind="ExternalInput")
    d_sm = nc.dram_tensor("smalls", [128, 9 * SGS], f32, kind="ExternalInput")
    d_c = nc.dram_tensor("coords", [4, 3, N], f32, kind="ExternalInput")
    d_out = nc.dram_tensor("out", [SGS, 2, 48, NH], f32, kind="ExternalOutput")

    with tile.TileContext(nc) as tc:
        with tc.tile_pool(name="const", bufs=1) as cp, \
             tc.tile_pool(name="acts", bufs=2) as ap, \
             tc.tile_pool(name="outp", bufs=4) as op_, \
             tc.tile_pool(name="psA", bufs=8, space="PSUM") as psA:

            # ---- persistent constants (w2 chunked so sg0 starts early) ----
            w1sb = cp.tile([128, 128 * SGS], f32, tag="w1")
            for a in range(4):
                nc.sync.dma_start(w1sb[32 * a:32 * a + 3, :], d_w1[a])
            w2sb = cp.tile([128, 128 * SGS], f32, tag="w2")
            q = 128 * SGS // 4
            for b in range(4):
                for ck in range(4):
                    nc.sync.dma_start(
                        w2sb[32 * b:32 * b + 32, q * ck:q * ck + q],
                        d_w2[b, :, q * ck:q * ck + q])
            w3sb = cp.tile([128, 48 * SGS], f32, tag="w3")
            for a in range(4):
                nc.sync.dma_start(w3sb[32 * a:32 * a + 32, :], d_w3[a])
            smalls = cp.tile([128, 9 * SGS], f32, tag="sm")
            nc.sync.dma_start(smalls[:], d_sm[:])
            c4 = cp.tile([128, N], f32, tag="c4")
            nc.vector.memset(c4[:], 0.0)  # rows 3-31 of each group MUST be 0
            for a in range(4):
                nc.sync.dma_start(c4[32 * a:32 * a + 3, :], d_c[a])
            magic = cp.tile([128, 1], f32, tag="mg")
            nc.vector.memset(magic[:], MAGIC)

            def front_A(sg, h):
                # ---- layer 1: 16 nets as 32x32 tiles, K=3 (w,b aug) ----
                PZ1 = []
                for a in range(4):
                    t = psA.tile([128, 512], f32, tag="P", name=f"pz1_{sg}_{h}_{a}")
                    PZ1.append(t)
                for a in range(4):
                    for b in range(4):
                        nc.tensor.matmul(
                            out=PZ1[a][32 * b:32 * b + 32, :],
                            lhsT=w1sb[32 * a:32 * a + 3,
                                      128 * sg + 32 * b:128 * sg + 32 * b + 32],
                            rhs=c4[32 * a:32 * a + 3, NH * h:NH * h + NH],
                            start=True, stop=True,
                            tile_position=(32 * a, 32 * b))
                Q1 = ap.tile([128, 2048], f32, tag="Q1")
                for a in range(4):
                    sl = slice(512 * a, 512 * a + 512)
                    nc.scalar.activation(Q1[:, sl], PZ1[a][:], AF.Identity,
                                         bias=magic[:], scale=1.0)
                    nc.vector.scalar_tensor_tensor(
                        Q1[:, sl], Q1[:, sl], MAGIC, PZ1[a][:],
                        ALU.subtract, ALU.subtract)
                H1 = ap.tile([128, 2048], f32, tag="H1")
                nc.scalar.activation(H1[:], Q1[:], AF.Sin,
                                     bias=0.0, scale=-TWO_PI)
                return H1

            def front_B1(sg, h, H1):
                # ---- layer 2: 16 nets as 32x32 tiles, K=32 ----
                PZ2 = []
                for bt in range(4):
                    t = psA.tile([128, 512], f32, tag="P", name=f"pz2_{sg}_{h}_{bt}")
                    PZ2.append(t)
                for bt in range(4):
                    for a in range(4):
                        nc.tensor.matmul(
                            out=PZ2[bt][32 * a:32 * a + 32, :],
                            lhsT=w2sb[32 * bt:32 * bt + 32,
                                      128 * sg + 32 * a:128 * sg + 32 * a + 32],
                            rhs=H1[32 * bt:32 * bt + 32,
                                   512 * a:512 * a + 512],
                            start=True, stop=True,
                            tile_position=(32 * bt, 32 * a))
                return PZ2

            def front_B2(sg, h, Q2, PZ2):
                for bt in range(4):
                    sl = slice(1024 * bt + 512 * h, 1024 * bt + 512 * h + 512)
                    nc.vector.tensor_scalar(
                        Q2[:, sl], PZ2[bt][:],
                        smalls[:, 9 * sg + bt:9 * sg + bt + 1], MAGIC,
                        ALU.add, ALU.add)
                    nc.vector.scalar_tensor_tensor(
                        Q2[:, sl], Q2[:, sl], MAGIC, PZ2[bt][:],
                        ALU.subtract, ALU.subtract)

            def back_sin2(sg, Q2, H2, bts=(0, 1, 2, 3)):
                for bt in bts:
                    nc.scalar.activation(
                        H2[:, 1024 * bt:1024 * bt + 1024],
                        Q2[:, 1024 * bt:1024 * bt + 1024],
                        AF.Sin,
                        bias=smalls[:, 9 * sg + 4 + bt:9 * sg + 4 + bt + 1],
                        scale=-TWO_PI)

            def back_out(sg, h, H2):
                PC = psA.tile([128, 512], f32, tag="P")
                for bt in range(4):
                    nc.tensor.matmul(
                        out=PC[32 * bt:32 * bt + 12, :],
                        lhsT=w3sb[:, 48 * sg + 12 * bt:48 * sg + 12 * bt + 12],
                        rhs=H2[:, 1024 * bt + 512 * h:1024 * bt + 512 * h + 512],
                        start=True, stop=True,
                        tile_position=(0, 32 * bt))
                OT = op_.tile([128, 512], f32, tag="OT")
                nc.scalar.activation(OT[:], PC[:], AF.Identity,
                                     bias=smalls[:, 9 * sg + 8:9 * sg + 9],
                                     scale=1.0)
                for bt in range(4):
                    nc.sync.dma_start(
                        d_out[sg, h, 12 * bt:12 * bt + 12, :],
                        OT[32 * bt:32 * bt + 12, :])

            # software-pipelined emission with deferred DVE tail:
            # per half t: L1(t) | B2(t-1) | back(sg-1) | Jt1/Jq1(t) | sin1(t) | L2(t)
            prev = None          # (sg, Q2, H2) of previous supergroup
    
            pending = None       # deferred front_B2 args of previous half
            for sg in range(SGS):
                Q2 = ap.tile([128, 4096], f32, tag="Q2")
                H2 = ap.tile([128, 4096], f32, tag="H2")
                for h in range(2):
                    H1 = front_A(sg, h)
                    if pending is not None:
                        front_B2(*pending)
                        pending = None
                    if h == 0 and prev is not None:
                        back_sin2(prev[0], prev[1], prev[2])
                    if prev is not None:
                        back_out(prev[0], h, prev[2])
                    PZ2 = front_B1(sg, h, H1)
                    pending = (sg, h, Q2, PZ2)
                prev = (sg, Q2, H2)
            front_B2(*pending)
            back_sin2(prev[0], prev[1], prev[2])
            back_out(prev[0], 0, prev[2])
            back_out(prev[0], 1, prev[2])

    nc.compile()
    return nc


def _prep_core_inputs(w1, b1, w2, b2, w3, b3, coords, core):
    s = np.float32(W0 / TWO_PI)
    B0 = core * BPC
    sl = slice(B0, B0 + BPC)

    # net (sg, a, b) = batch B0 + 16sg + 4a + b
    w1c = w1[sl, :, :, 0].reshape(SGS, 4, 4, IN, H)
    b1c = b1[sl, :, 0].reshape(SGS, 4, 4, H)
    aug = np.concatenate([w1c, b1c[:, :, :, None, :]], axis=3) * s  # [sg,a,b,3,32]
    w1aug = np.ascontiguousarray(
        aug.transpose(1, 3, 0, 2, 4).reshape(4, 3, SGS * 128)).astype(np.float32)

    # L2: partition 32bt+i, free 32a+o  (net 4a+bt)
    w2c = (w2[sl, :, :, 0] * s).reshape(SGS, 4, 4, H, H)  # [sg,a,b,i,o]
    w2s = np.ascontiguousarray(
        w2c.transpose(2, 3, 0, 1, 4).reshape(4, 32, SGS * 128)).astype(np.float32)

    # L3 block-diag per (sg, bt): [128, 12]: [32a+i, 3a'+c] = w3[net(sg,a,bt)]
    w3c = w3[sl, :, :, 0].reshape(SGS, 4, 4, H, OUT)  # [sg,a,b,i,c]
    blk = np.zeros((SGS, 4, 4, H, 4, OUT), np.float32)  # [sg,a,b,i,a',c]
    for a in range(4):
        blk[:, a, :, :, a, :] = w3c[:, a]
    # free inside sg block: 12*bt + 3*a' + c ; partition 32*a + i
    w3blk = np.ascontiguousarray(
        blk.transpose(1, 3, 0, 2, 4, 5).reshape(4, 32, SGS * 48)).astype(np.float32)

    b2c = b2[sl, :, 0].reshape(SGS, 4, 4, H)  # [sg,a,b,o]
    b3c = b3[sl, :, 0].reshape(SGS, 4, 4, OUT)  # [sg,a,b,c]
    smalls = np.zeros((128, SGS, 9), np.float32)
    p = np.arange(128)
    a_idx, o_idx = p // 32, p % 32
    for bt in range(4):
        # L2 psum partition 32a+o, segment bt -> net 4a+bt
        smalls[:, :, bt] = (b2c[:, a_idx, bt, o_idx] * s).T
        smalls[:, :, 4 + bt] = (b2c[:, a_idx, bt, o_idx] * np.float32(W0)).T
    # b3: PC partition 32bt + 3a + c -> net 4a+bt
    bt_idx, m_idx = p // 32, p % 32
    a3, c3 = m_idx // 3, m_idx % 3
    for pi in range(128):
        if m_idx[pi] < 12:
            smalls[pi, :, 8] = b3c[:, a3[pi], bt_idx[pi], c3[pi]]
    smalls = np.ascontiguousarray(smalls.reshape(128, SGS * 9))

    ch = np.zeros((4, 3, N), np.float32)
    ch[:, :IN, :] = coords.T[None, :, :]
    ch[:, IN, :] = 1.0

    return {"w1aug": w1aug, "w2s": w2s, "w3blk": w3blk,
            "smalls": smalls, "coords": ch}


def _unshard(res_list):
    outs = []
    for r in res_list:
        o = r["out"].reshape(SGS, 2, 4, 4, OUT, NH)      # [sg,h,bt,a,c,n]
        o = o.transpose(0, 3, 2, 1, 5, 4)                # [sg,a,bt,h,n,c]
        outs.append(np.ascontiguousarray(o.reshape(BPC, N, OUT)))
    return np.concatenate(outs, axis=0)


def _run(inputs, trace=False, trace_kwargs=None):
    global _compiled
    if _compiled is None:
        _compiled = _build_module()
    nc = _compiled
    arrs = {k: np.asarray(v, dtype=np.float32) for k, v in inputs.items()}
    in_maps = [_prep_core_inputs(arrs["w1"], arrs["b1"], arrs["w2"], arrs["b2"],
                                 arrs["w3"], arrs["b3"], arrs["coords"], c)
               for c in range(N_CORES)]
    kw = {}
    if trace:
        kw["trace"] = True
        if trace_kwargs:
            kw.update(trace_kwargs)
    res = bass_utils.run_bass_kernel_spmd(nc, in_maps, core_ids=list(range(N_CORES)),
                                          **kw)
    out = _unshard(res.results)
    return out, res


def kernel(**inputs):
    out, _ = _run(inputs, trace=False)
    return out


# revision 14
# speedup vs baseline: 1.0470x; 1.0012x over previous
"""BatchSiren Trainium2 kernel.

B=2048 independent SIREN MLPs (2->32->32->3, sin activations, w0=30),
each evaluated on the same N=1024 coordinate grid.

Strategy (pure data parallel over 8 cores, 256 nets/core):
- 16 supergroups of 16 nets per core. Nets packed 16-at-a-time onto the
  128x128 PE array via 32x32 tile_position (independent concurrent tiles),
  fp32 throughout.
- All activations stay in [feature-partition, points-free] layout.
- sin(w0*z): weights pre-scaled by w0/2pi on host so matmuls produce
  arguments in CYCLE units; range reduction to one period via the
  magic-number round trick (ACT Identity+MAGIC / DVE tensor_scalar), then
  ACT Sin with scale=-2pi maps back to radians (Sin table covers [-pi,pi]).
- Layer-3 output is produced point-major ([points, 4nets x 3ch]) via
  col-tiled matmuls with block-diagonal w3, so the final DMA writes
  2KB-contiguous runs per partition.
"""
import numpy as np

import concourse.bacc as bacc
import concourse.bass as bass
import concourse.mybir as mybir
import concourse.tile as tile
from concourse import bass_utils

f32 = mybir.dt.float32
AF = mybir.ActivationFunctionType
ALU = mybir.AluOpType

W0 = 30.0
MAGIC = float(1.5 * 2 ** 23)
TWO_PI = float(2.0 * np.pi)
N_CORES = 8
B, N, IN, H, OUT = 2048, 1024, 2, 32, 3
BPC = B // N_CORES        # 256 nets per core
SGS = BPC // 16           # 16 supergroups of 16 nets
NH = N // 2               # 512 points per half

_compiled = None


def _build_module():
    nc = bacc.Bacc("TRN2", target_bir_lowering=False, debug=False)

    d_w1 = nc.dram_tensor("w1aug", [4, 3, 128 * SGS], f32, kind="ExternalInput")
    d_w2 = nc.dram_tensor("w2s", [4, 32, 128 * SGS], f32, kind="ExternalInput")
    d_w3 = nc.dram_tensor("w3blk", [4, 32, 48 * SGS], f32, kind="ExternalInput")
    d_sm = nc.dram_tensor("smalls", [128, 9 * SGS], f32, kind="ExternalInput")
    d_c = nc.dram_tensor("coords", [4, 3, N], f32, kind="ExternalInput")
    d_out = nc.dram_tensor("out", [SGS, 2, 48, NH], f32, kind="ExternalOutput")

    with tile.TileContext(nc, pool_alloc_mode="queue") as tc:
        with tc.tile_pool(name="const", bufs=1) as cp, \
             tc.tile_pool(name="acts", bufs=2) as ap, \
             tc.tile_pool(name="outp", bufs=4) as op_, \
             tc.tile_pool(name="psA", bufs=8, space="PSUM") as psA:

            # ---- persistent constants (w2 chunked so sg0 starts early) ----
            w1sb = cp.tile([128, 128 * SGS], f32, tag="w1")
            for a in range(4):
                nc.sync.dma_start(w1sb[32 * a:32 * a + 3, :], d_w1[a])
            w2sb = cp.tile([128, 128 * SGS], f32, tag="w2")
            q = 128 * SGS // 4
            for b in range(4):
                for ck in range(4):
                    nc.sync.dma_start(
                        w2sb[32 * b:32 * b + 32, q * ck:q * ck + q],
                        d_w2[b, :, q * ck:q * ck + q])
            w3sb = cp.tile([128, 48 * SGS], f32, tag="w3")
            for a in range(4):
                nc.sync.dma_start(w3sb[32 * a:32 * a + 32, :], d_w3[a])
            smalls = cp.tile([128, 9 * SGS], f32, tag="sm")
            nc.sync.dma_start(smalls[:], d_sm[:])
            c4 = cp.tile([128, N], f32, tag="c4")
            nc.vector.memset(c4[:], 0.0)  # rows 3-31 of each group MUST be 0
            for a in range(4):
                nc.sync.dma_start(c4[32 * a:32 * a + 3, :], d_c[a])
            magic = cp.tile([128, 1], f32, tag="mg")
            nc.vector.memset(magic[:], MAGIC)

            def front_A(sg, h):
                # ---- layer 1: 16 nets as 32x32 tiles, K=3 (w,b aug) ----
                PZ1 = []
                for a in range(4):
                    t = psA.tile([128, 512], f32, tag="P", name=f"pz1_{sg}_{h}_{a}")
                    PZ1.append(t)
                for a in range(4):
                    for b in range(4):
                        nc.tensor.matmul(
                            out=PZ1[a][32 * b:32 * b + 32, :],
                            lhsT=w1sb[32 * a:32 * a + 3,
                                      128 * sg + 32 * b:128 * sg + 32 * b + 32],
                            rhs=c4[32 * a:32 * a + 3, NH * h:NH * h + NH],
                            start=True, stop=True,
                            tile_position=(32 * a, 32 * b))
                Q1 = ap.tile([128, 2048], f32, tag="Q1")
                for a in range(4):
                    sl = slice(512 * a, 512 * a + 512)
                    nc.scalar.activation(Q1[:, sl], PZ1[a][:], AF.Identity,
                                         bias=magic[:], scale=1.0)
                    nc.vector.scalar_tensor_tensor(
                        Q1[:, sl], Q1[:, sl], MAGIC, PZ1[a][:],
                        ALU.subtract, ALU.subtract)
                H1 = ap.tile([128, 2048], f32, tag="H1")
                nc.scalar.activation(H1[:], Q1[:], AF.Sin,
                                     bias=0.0, scale=-TWO_PI)
                return H1

            def front_B1(sg, h, H1):
                # ---- layer 2: 16 nets as 32x32 tiles, K=32 ----
                PZ2 = []
                for bt in range(4):
                    t = psA.tile([128, 512], f32, tag="P", name=f"pz2_{sg}_{h}_{bt}")
                    PZ2.append(t)
                for bt in range(4):
                    for a in range(4):
                        nc.tensor.matmul(
                            out=PZ2[bt][32 * a:32 * a + 32, :],
                            lhsT=w2sb[32 * bt:32 * bt + 32,
                                      128 * sg + 32 * a:128 * sg + 32 * a + 32],
                            rhs=H1[32 * bt:32 * bt + 32,
                                   512 * a:512 * a + 512],
                            start=True, stop=True,
                            tile_position=(32 * bt, 32 * a))
                return PZ2

            def front_B2(sg, h, Q2, PZ2):
                for bt in range(4):
                    sl = slice(1024 * bt + 512 * h, 1024 * bt + 512 * h + 512)
                    nc.vector.tensor_scalar(
                        Q2[:, sl], PZ2[bt][:],
                        smalls[:, 9 * sg + bt:9 * sg + bt + 1], MAGIC,
                        ALU.add, ALU.add)
                    nc.vector.scalar_tensor_tensor(
                        Q2[:, sl], Q2[:, sl], MAGIC, PZ2[bt][:],
                        ALU.subtract, ALU.subtract)

            def back_sin2(sg, Q2, H2, bts=(0, 1, 2, 3)):
                for bt in bts:
                    nc.scalar.activation(
                        H2[:, 1024 * bt:1024 * bt + 1024],
                        Q2[:, 1024 * bt:1024 * bt + 1024],
                        AF.Sin,
                        bias=smalls[:, 9 * sg + 4 + bt:9 * sg + 4 + bt + 1],
                        scale=-TWO_PI)

            def back_out(sg, h, H2):
                PC = psA.tile([128, 512], f32, tag="P")
                for bt in range(4):
                    nc.tensor.matmul(
                        out=PC[32 * bt:32 * bt + 12, :],
                        lhsT=w3sb[:, 48 * sg + 12 * bt:48 * sg + 12 * bt + 12],
                        rhs=H2[:, 1024 * bt + 512 * h:1024 * bt + 512 * h + 512],
                        start=True, stop=True,
                        tile_position=(0, 32 * bt))
                OT = op_.tile([128, 512], f32, tag="OT")
                nc.scalar.activation(OT[:], PC[:], AF.Identity,
                                     bias=smalls[:, 9 * sg + 8:9 * sg + 9],
                                     scale=1.0)
                for bt in range(4):
                    nc.sync.dma_start(
                        d_out[sg, h, 12 * bt:12 * bt + 12, :],
                        OT[32 * bt:32 * bt + 12, :])

            # software-pipelined emission with deferred DVE tail:
            # per half t: L1(t) | B2(t-1) | back(sg-1) | Jt1/Jq1(t) | sin1(t) | L2(t)
            prev = None          # (sg, Q2, H2) of previous supergroup
    
            pending = None       # deferred front_B2 args of previous half
            for sg in range(SGS):
                Q2 = ap.tile([128, 4096], f32, tag="Q2")
                H2 = ap.tile([128, 4096], f32, tag="H2")
                for h in range(2):
                    H1 = front_A(sg, h)
                    if pending is not None:
                        front_B2(*pending)
                        pending = None
                    if h == 0 and prev is not None:
                        back_sin2(prev[0], prev[1], prev[2])
                    if prev is not None:
                        back_out(prev[0], h, prev[2])
                    PZ2 = front_B1(sg, h, H1)
                    pending = (sg, h, Q2, PZ2)
                prev = (sg, Q2, H2)
            front_B2(*pending)
            back_sin2(prev[0], prev[1], prev[2])
            back_out(prev[0], 0, prev[2])
            back_out(prev[0], 1, prev[2])

    nc.compile()
    return nc


def _prep_core_inputs(w1, b1, w2, b2, w3, b3, coords, core):
    s = np.float32(W0 / TWO_PI)
    B0 = core * BPC
    sl = slice(B0, B0 + BPC)

    # net (sg, a, b) = batch B0 + 16sg + 4a + b
    w1c = w1[sl, :, :, 0].reshape(SGS, 4, 4, IN, H)
    b1c = b1[sl, :, 0].reshape(SGS, 4, 4, H)
    aug = np.concatenate([w1c, b1c[:, :, :, None, :]], axis=3) * s  # [sg,a,b,3,32]
    w1aug = np.ascontiguousarray(
        aug.transpose(1, 3, 0, 2, 4).reshape(4, 3, SGS * 128)).astype(np.float32)

    # L2: partition 32bt+i, free 32a+o  (net 4a+bt)
    w2c = (w2[sl, :, :, 0] * s).reshape(SGS, 4, 4, H, H)  # [sg,a,b,i,o]
    w2s = np.ascontiguousarray(
        w2c.transpose(2, 3, 0, 1, 4).reshape(4, 32, SGS * 128)).astype(np.float32)

    # L3 block-diag per (sg, bt): [128, 12]: [32a+i, 3a'+c] = w3[net(sg,a,bt)]
    w3c = w3[sl, :, :, 0].reshape(SGS, 4, 4, H, OUT)  # [sg,a,b,i,c]
    blk = np.zeros((SGS, 4, 4, H, 4, OUT), np.float32)  # [sg,a,b,i,a',c]
    for a in range(4):
        blk[:, a, :, :, a, :] = w3c[:, a]
    # free inside sg block: 12*bt + 3*a' + c ; partition 32*a + i
    w3blk = np.ascontiguousarray(
        blk.transpose(1, 3, 0, 2, 4, 5).reshape(4, 32, SGS * 48)).astype(np.float32)

    b2c = b2[sl, :, 0].reshape(SGS, 4, 4, H)  # [sg,a,b,o]
    b3c = b3[sl, :, 0].reshape(SGS, 4, 4, OUT)  # [sg,a,b,c]
    smalls = np.zeros((128, SGS, 9), np.float32)
    p = np.arange(128)
    a_idx, o_idx = p // 32, p % 32
    for bt in range(4):
        # L2 psum partition 32a+o, segment bt -> net 4a+bt
        smalls[:, :, bt] = (b2c[:, a_idx, bt, o_idx] * s).T
        smalls[:, :, 4 + bt] = (b2c[:, a_idx, bt, o_idx] * np.float32(W0)).T
    # b3: PC partition 32bt + 3a + c -> net 4a+bt
    bt_idx, m_idx = p // 32, p % 32
    a3, c3 = m_idx // 3, m_idx % 3
    for pi in range(128):
        if m_idx[pi] < 12:
            smalls[pi, :, 8] = b3c[:, a3[pi], bt_idx[pi], c3[pi]]
    smalls = np.ascontiguousarray(smalls.reshape(128, SGS * 9))

    ch = np.zeros((4, 3, N), np.float32)
    ch[:, :IN, :] = coords.T[None, :, :]
    ch[:, IN, :] = 1.0

    return {"w1aug": w1aug, "w2s": w2s, "w3blk": w3blk,
            "smalls": smalls, "coords": ch}


def _unshard(res_list):
    outs = []
    for r in res_list:
        o = r["out"].reshape(SGS, 2, 4, 4, OUT, NH)      # [sg,h,bt,a,c,n]
        o = o.transpose(0, 3, 2, 1, 5, 4)                # [sg,a,bt,h,n,c]
        outs.append(np.ascontiguousarray(o.reshape(BPC, N, OUT)))
    return np.concatenate(outs, axis=0)


def _run(inputs, trace=False, trace_kwargs=None):
    global _compiled
    if _compiled is None:
        _compiled = _build_module()
    nc = _compiled
    arrs = {k: np.asarray(v, dtype=np.float32) for k, v in inputs.items()}
    in_maps = [_prep_core_inputs(arrs["w1"], arrs["b1"], arrs["w2"], arrs["b2"],
                                 arrs["w3"], arrs["b3"], arrs["coords"], c)
               for c in range(N_CORES)]
    kw = {}
    if trace:
        kw["trace"] = True
        if trace_kwargs:
            kw.update(trace_kwargs)
    res = bass_utils.run_bass_kernel_spmd(nc, in_maps, core_ids=list(range(N_CORES)),
                                          **kw)
    out = _unshard(res.results)
    return out, res


def kernel(**inputs):
    out, _ = _run(inputs, trace=False)
    return out
